# revision 8
# baseline (speedup 1.0000x reference)
"""DeCAN attention TRN2 kernel: 8-core head-parallel, transfer-optimized.

The end-to-end metric is dominated by host<->device bytes shipped per
execution (~12.5 GB/s through the axon tunnel), so the kernel minimizes
wire traffic:
  * every tensor crosses the wire in fp16 (rel-err budget is 2e-2;
    fp16 keeps us ~2 orders of magnitude under it),
  * no byte is shipped twice: inputs shared by several cores are shipped
    as disjoint 1/N slices and AllGathered on device
      - cos/sin swizzles: 8-rank AG,
      - hidden (per batch): 4-rank AG over [[0-3],[4-7]],
      - Wq / Wkv / Wo head-group slices: 2-rank AG over [[0,4],..,[3,7]],
    while per-core-exclusive prev_k/prev_v head slices ship directly,
  * the output partial sums are reduced on device with two 4-rank
    ReduceScatters (one per q-tile), so each core returns a disjoint
    [512, 1024] fp16 shard instead of a full [2048, 1024] fp32 partial.

Sharding (as before): core c handles batch b = c//4 and 8 q-heads
{g+4j, j=0..7} with g = c%4.  Each q-head h attends to stacked-KV head h
(prev heads 0..27 shipped, head g+28 projected on device).  Compute is
fp16 on the PE (2x rate) with fp32 PSUM accumulation:
  A) prev_k RoPE on DVE (pair-interleaved d order so rotate-half is an
     adjacent-partition stream_shuffle); fused [Wk|Wv] projection for the
     new head; V^T transposed to k-major via PE transpose; Q^T = RoPE of
     Wq^T.T @ hidden^T
  B) per (q-tile, head): S^T blocks on PE, exp via ACT (scale=1/8),
     causal blocks zeroed by gpsimd affine_select, O^T = V_aug.T @ P^T
     with a free rowsum row, normalization via reciprocal + ones-matmul
     partition broadcast
  C) out^T = Wo^T.T @ O^T_cat per q-tile -> [H, L] fp16 partial in DRAM
     -> ReduceScatter(add) over the 4 cores of the batch.
"""

import numpy as np
from contextlib import ExitStack

import concourse.bass as bass
from concourse import bacc
import concourse.mybir as mybir
import concourse.tile as tile
from concourse.bass_utils import run_bass_kernel_spmd

B, L, H, HD, NK, NQ = 2, 1024, 2048, 64, 4, 32
NPREV = NQ - NK
NCORES = 8
HPC = NQ // 4          # 8 heads per core
QT = 512               # q tile (moving dim)
NQT = L // QT          # 2
KT = 128               # k tile
NKT = L // KT          # 8
ET = 128
NET = H // ET          # 16

F16 = mybir.dt.float16
F32 = mybir.dt.float32

# pair-interleaved d order: rotate-half partner adjacent
DPERM = np.empty(HD, np.int64)
DPERM[0::2] = np.arange(0, HD // 2)
DPERM[1::2] = np.arange(HD // 2, HD)
SWAP_MASK = [p ^ 1 for p in range(32)]

QUADS = [[0, 1, 2, 3], [4, 5, 6, 7]]
PAIRS = [[0, 4], [1, 5], [2, 6], [3, 7]]


# row placement of head-slot j inside the 4 [128 x L] q/k tiles.
# j7 is the device-projected new head; it must sit at rows 0:64 of tile 3
# (PSUM results land on partitions 0:63), so tile 3 is [j7 | j6].
def qk_row(j):
    if j < 6:
        return j // 2, 64 * (j % 2)
    return 3, 0 if j == 7 else 64


def _classify(mask2d):
    """mask2d: [L(q), L(k)] bool -> block classes + list of arbitrary blocks."""
    classes = {}
    arb = []
    for qt in range(NQT):
        for kt in range(NKT):
            sub = mask2d[qt * QT:(qt + 1) * QT, kt * KT:(kt + 1) * KT]
            if sub.all():
                classes[(qt, kt)] = "full"
            elif not sub.any():
                classes[(qt, kt)] = "skip"
            else:
                qi = np.arange(qt * QT, (qt + 1) * QT)[:, None]
                ki = np.arange(kt * KT, (kt + 1) * KT)[None, :]
                if (sub == (qi >= ki)).all():
                    classes[(qt, kt)] = "diag"
                else:
                    classes[(qt, kt)] = "arb"
                    arb.append((qt, kt))
    return classes, arb


def build_program(classes, arb):
    arb_idx = {blk: i for i, blk in enumerate(arb)}
    nc = bacc.Bacc(num_devices=NCORES)
    hx8 = nc.declare_dram_parameter("hx8", [32, NET * L], F16, isOutput=False)
    wqh = nc.declare_dram_parameter("wqh", [256, H], F16, isOutput=False)
    wkvh = nc.declare_dram_parameter("wkvh", [64, H], F16, isOutput=False)
    woh = nc.declare_dram_parameter("woh", [256, H], F16, isOutput=False)
    csh = nc.declare_dram_parameter("csh", [32, L], F16, isOutput=False)
    pk = nc.declare_dram_parameter("pk", [448, L], F16, isOutput=False)
    pv = nc.declare_dram_parameter("pv", [128, NKT * 7 * (HD + 1)], F16, isOutput=False)
    id64 = nc.declare_dram_parameter("id64", [64, 64], F32, isOutput=False)
    maskf = None
    if arb:
        maskf = nc.declare_dram_parameter("maskf", [len(arb), KT, QT], F16, isOutput=False)
    outp = nc.declare_dram_parameter("outp", [4 * ET, L], F16, isOutput=True)

    with ExitStack() as ctx:
        ctx.enter_context(nc.allow_low_precision(reason="fp16 compute"))
        tc = ctx.enter_context(tile.TileContext(nc))

        dram = ctx.enter_context(tc.tile_pool(name="dram", bufs=1, space="DRAM"))
        const = ctx.enter_context(tc.tile_pool(name="const", bufs=1))
        persist = ctx.enter_context(tc.tile_pool(name="persist", bufs=1))

        # ---- bounce inputs to internal DRAM and launch AllGathers --------
        cs_b = dram.tile([32, L], F16)
        nc.scalar.dma_start(out=cs_b, in_=csh[:, :])
        hx_b = dram.tile([32, NET * L], F16)
        nc.sync.dma_start(out=hx_b, in_=hx8[:, :])
        wkv_b = dram.tile([64, H], F16)
        nc.scalar.dma_start(out=wkv_b, in_=wkvh[:, :])
        wq_b = dram.tile([256, H], F16)
        nc.gpsimd.dma_start(out=wq_b, in_=wqh[:, :])
        wo_b = dram.tile([256, H], F16)
        nc.gpsimd.dma_start(out=wo_b, in_=woh[:, :])

        csf = dram.tile([256, L], F16, addr_space="Shared")
        nc.gpsimd.collective_compute(
            "AllGather", mybir.AluOpType.bypass, replica_groups=[list(range(8))],
            ins=[cs_b[:]], outs=[csf[:]])
        hxall = dram.tile([128, NET * L], F16)
        nc.gpsimd.collective_compute(
            "AllGather", mybir.AluOpType.bypass, replica_groups=QUADS,
            ins=[hx_b[:]], outs=[hxall[:]])
        wkvf = dram.tile([128, H], F16)
        nc.gpsimd.collective_compute(
            "AllGather", mybir.AluOpType.bypass, replica_groups=PAIRS,
            ins=[wkv_b[:]], outs=[wkvf[:]])
        wqf = dram.tile([512, H], F16)
        nc.gpsimd.collective_compute(
            "AllGather", mybir.AluOpType.bypass, replica_groups=PAIRS,
            ins=[wq_b[:]], outs=[wqf[:]])
        wof = dram.tile([512, H], F16)
        nc.gpsimd.collective_compute(
            "AllGather", mybir.AluOpType.bypass, replica_groups=PAIRS,
            ins=[wo_b[:]], outs=[wof[:]])

        partials = [dram.tile([H, QT], F16, name=f"partial{qt}")
                    for qt in range(NQT)]
        rsouts = [dram.tile([H // 4, QT], F16, name=f"rsout{qt}")
                  for qt in range(NQT)]

        # ---- constants ----------------------------------------------------
        cos16 = const.tile([128, L], F16)
        nc.scalar.dma_start(out=cos16, in_=csf[0:128, :])
        sin16 = const.tile([128, L], F16)
        nc.scalar.dma_start(out=sin16, in_=csf[128:256, :])
        cos32 = const.tile([128, L], F32)
        nc.vector.tensor_copy(cos32, cos16)
        sin32 = const.tile([128, L], F32)
        nc.vector.tensor_copy(sin32, sin16)
        ones1 = const.tile([128, 64], F16)
        nc.vector.memset(ones1, 1.0)
        id64t = const.tile([64, 64], F32)
        nc.scalar.dma_start(out=id64t, in_=id64[:, :])

        qTt = persist.tile([128, 4, L], F16, tag="qT")
        kTt = persist.tile([128, 4, L], F16, tag="kT")
        vaugt = persist.tile([128, NKT, HPC, HD + 1], F16, tag="vaug")
        oTt = persist.tile([128, 4, L], F16, tag="oT")

        # ---------------- phase A: projections + RoPE + V staging ----------
        with ExitStack() as actx:
            pa = actx.enter_context(tc.tile_pool(name="phaseA", bufs=1))
            u_p = actx.enter_context(tc.tile_pool(name="ropeu", bufs=2))
            t2_p = actx.enter_context(tc.tile_pool(name="ropet2", bufs=2))
            psA = actx.enter_context(tc.tile_pool(name="psA", bufs=2, space="PSUM"))

            kpre = pa.tile([128, 4, L], F16, tag="kpre")
            for t in range(3):
                nc.scalar.dma_start(out=kpre[:, t, :],
                                    in_=pk[128 * t:128 * (t + 1), :])
            nc.scalar.dma_start(out=kpre[64:128, 3, :], in_=pk[384:448, :])

            wkvt = pa.tile([128, NET, 128], F16, tag="wkv")
            nc.sync.dma_start(
                out=wkvt, in_=wkvf[:, :].rearrange("p (et m) -> p et m", m=128))
            hxt = pa.tile([128, NET, L], F16, tag="hx")
            for g2 in range(8):
                nc.sync.dma_start(
                    out=hxt[:, 2 * g2:2 * (g2 + 1), :],
                    in_=hxall[:, 2 * g2 * L:2 * (g2 + 1) * L]
                    .rearrange("p (et l) -> p et l", l=L))

            def rope16(dst, src, rows, lt):
                """fp16 RoPE for prev heads: dst = src*cos + shuffle(src)*sinPre."""
                r0, r1 = rows
                ls = slice(lt * QT, (lt + 1) * QT)
                u = u_p.tile([128, QT], F16, tag="u16", name="u16")
                t2 = t2_p.tile([128, QT], F16, tag="t216", name="t216")
                nc.vector.stream_shuffle(u[r0:r1, :], src, SWAP_MASK)
                nc.vector.tensor_mul(u[r0:r1, :], u[r0:r1, :], sin16[r0:r1, ls])
                nc.gpsimd.tensor_mul(t2[r0:r1, :], src, cos16[r0:r1, ls])
                nc.vector.tensor_add(dst, u[r0:r1, :], t2[r0:r1, :])

            def rope32(dst, src, rows, lt):
                """fp32-in (PSUM) RoPE, fp16 out."""
                r0, r1 = rows
                ls = slice(lt * QT, (lt + 1) * QT)
                u = u_p.tile([128, QT], F32, tag="u32", name="u32")
                t2 = t2_p.tile([128, QT], F32, tag="t232", name="t232")
                nc.vector.stream_shuffle(u[r0:r1, :], src, SWAP_MASK)
                nc.vector.tensor_mul(u[r0:r1, :], u[r0:r1, :], sin32[r0:r1, ls])
                nc.vector.tensor_mul(t2[r0:r1, :], src, cos32[r0:r1, ls])
                nc.vector.tensor_add(dst, u[r0:r1, :], t2[r0:r1, :])

            # prev_k heads: RoPE from DMA'd tiles (no PE dependency)
            for t in (0, 1, 2, 3):
                rows = (0, 128) if t < 3 else (64, 128)
                for lt in range(NQT):
                    ls = slice(lt * QT, (lt + 1) * QT)
                    rope16(kTt[rows[0]:rows[1], t, ls], kpre[rows[0]:rows[1], t, ls],
                           rows, lt)

            # Wq streamed per m-tile (2 rotating slots)
            wq_tiles = [None] * 4
            for mt in (0, 1, 2, 3):
                wqmt = pa.tile([128, NET, 128], F16, tag="wqmt", bufs=2,
                               name=f"wqmt{mt}")
                nc.sync.dma_start(
                    out=wqmt,
                    in_=wqf[128 * mt:128 * (mt + 1), :]
                    .rearrange("p (et m) -> p et m", m=128))
                wq_tiles[mt] = wqmt

            # fused new-head K/V projection (psum rows 0:63 = K^T perm'd d,
            # rows 64:127 = V^T natural d) interleaved with the first Q m-tile
            vT = pa.tile([64, L], F32, tag="vT")
            for lt in range(NQT):
                ls = slice(lt * QT, (lt + 1) * QT)
                pskv = psA.tile([128, QT], F32, tag="pskv")
                psq0 = psA.tile([128, QT], F32, tag="psq0")
                for et in range(NET):
                    nc.tensor.matmul(pskv, wkvt[:, et, :], hxt[:, et, ls],
                                     start=(et == 0), stop=(et == NET - 1))
                    nc.tensor.matmul(psq0, wq_tiles[0][:, et, :], hxt[:, et, ls],
                                     start=(et == 0), stop=(et == NET - 1))
                rope32(kTt[0:64, 3, ls], pskv[0:64, :], (0, 64), lt)
                nc.vector.tensor_copy(vT[:, ls], pskv[64:128, :])
                rope32(qTt[:, 0, ls], psq0, (0, 128), lt)

            # transpose V^T [64, L] -> k-major V in vaug via PE transpose
            for ltk in range(NKT):
                psvt = psA.tile([128, HD], F32, tag="psvt", bufs=2)
                nc.tensor.transpose(
                    psvt, vT[:, ltk * 128:(ltk + 1) * 128], id64t)
                nc.vector.tensor_copy(vaugt[:, ltk, 7, 0:HD], psvt)

            # deferred bulk DMAs (needed only by phase B)
            nc.scalar.dma_start(
                out=vaugt[:, :, 0:7, :],
                in_=pv[:, :].rearrange("p (kt j d) -> p kt j d", kt=NKT, j=7))
            nc.gpsimd.memset(vaugt[:, :, 7, HD:HD + 1], 1.0)
            # remaining Q m-tiles
            for mt in range(1, 4):
                for lt in range(NQT):
                    ls = slice(lt * QT, (lt + 1) * QT)
                    psq = psA.tile([128, QT], F32, tag="psq")
                    for et in range(NET):
                        nc.tensor.matmul(
                            psq, wq_tiles[mt][:, et, :], hxt[:, et, ls],
                            start=(et == 0), stop=(et == NET - 1))
                    rope32(qTt[:, mt, ls], psq, (0, 128), lt)

        # ---------------- phase B: attention (+ phase C per q-tile) -------
        with ExitStack() as bctx:
            pb = bctx.enter_context(tc.tile_pool(name="phaseB", bufs=1))
            pt_p = bctx.enter_context(tc.tile_pool(name="pt", bufs=8))
            r_p = bctx.enter_context(tc.tile_pool(name="rsum", bufs=3))
            ob_p = bctx.enter_context(tc.tile_pool(name="obuf", bufs=3))
            psB = bctx.enter_context(tc.tile_pool(name="psB", bufs=3, space="PSUM"))
            psO = bctx.enter_context(tc.tile_pool(name="psO", bufs=2, space="PSUM"))
            psR = bctx.enter_context(tc.tile_pool(name="psR", bufs=1, space="PSUM"))
            psC = bctx.enter_context(tc.tile_pool(name="psC", bufs=2, space="PSUM"))

            maskts = []
            for i in range(len(arb)):
                mt_ = pb.tile([KT, QT], F16, tag=f"mask{i}", name=f"maskt{i}")
                nc.scalar.dma_start(out=mt_, in_=maskf[i, :, :])
                maskts.append(mt_)
            # prefetch Wo during attention
            wot = pb.tile([128, 4, H], F16, tag="wo")
            nc.scalar.dma_start(
                out=wot, in_=wof[:, :].rearrange("(ht p) e -> p ht e", p=128))

            for qt in range(NQT):
                qs = slice(qt * QT, (qt + 1) * QT)
                allowed = [kt for kt in range(NKT) if classes[(qt, kt)] != "skip"]
                for j in range(HPC):
                    pt_tile, base = qk_row(j)
                    rs = slice(base, base + 64)
                    op_, obase = j // 2, 64 * (j % 2)
                    pts = {}
                    for kt in allowed:
                        pss = psB.tile([128, QT], F32)
                        nc.tensor.matmul(
                            pss,
                            kTt[rs, pt_tile, kt * KT:(kt + 1) * KT],
                            qTt[rs, pt_tile, qs],
                            start=True, stop=True)
                        pt = pt_p.tile([128, QT], F16)
                        nc.scalar.activation(pt, pss,
                                             mybir.ActivationFunctionType.Exp,
                                             scale=float(HD) ** -0.5)
                        cls = classes[(qt, kt)]
                        if cls == "diag":
                            dbase = qt * QT - kt * KT
                            nc.gpsimd.affine_select(
                                out=pt, in_=pt, pattern=[[1, QT]],
                                compare_op=mybir.AluOpType.is_ge, fill=0.0,
                                base=dbase, channel_multiplier=-1)
                        elif cls == "arb":
                            nc.vector.tensor_mul(pt, pt, maskts[arb_idx[(qt, kt)]])
                        pts[kt] = pt
                    pso = psO.tile([HD + 1, QT], F32)
                    for i, kt in enumerate(allowed):
                        nc.tensor.matmul(pso, vaugt[:, kt, j, :], pts[kt],
                                         start=(i == 0), stop=(i == len(allowed) - 1))
                    r1 = r_p.tile([65, QT], F16)
                    nc.vector.reciprocal(r1[64:65, :], pso[64:65, :])
                    psr = psR.tile([64, QT], F32)
                    nc.tensor.matmul(psr, ones1[64:65, 0:64], r1[64:65, :],
                                     start=True, stop=True)
                    rbc = r_p.tile([64, QT], F32, tag="rbc")
                    nc.vector.tensor_copy(rbc, psr)
                    nc.vector.tensor_mul(oTt[obase:obase + 64, op_, qs],
                                         pso[0:64, :], rbc)

                # phase C for this q-tile: overlaps the other q-tile's attention
                for mt in range(NET):
                    pse = psC.tile([128, QT], F32)
                    for ht in range(4):
                        nc.tensor.matmul(pse, wot[:, ht, mt * 128:(mt + 1) * 128],
                                         oTt[:, ht, qs],
                                         start=(ht == 0), stop=(ht == 3))
                    ob = ob_p.tile([128, QT], F16)
                    if qt == 0:
                        nc.vector.tensor_copy(ob, pse)
                    else:
                        nc.scalar.copy(ob, pse)
                    nc.sync.dma_start(
                        out=partials[qt][mt * 128:(mt + 1) * 128, :], in_=ob)
                nc.gpsimd.collective_compute(
                    "ReduceScatter", mybir.AluOpType.add, replica_groups=QUADS,
                    ins=[partials[qt][:]], outs=[rsouts[qt][:]])
                nc.scalar.dma_start(out=outp[:, qs], in_=rsouts[qt][:])

    nc.finalize()
    return nc


_PROGRAM_CACHE = {}
_LAST = {}


def kernel(hidden_states, prev_k, prev_v, Wq, Wk, Wv, Wo, cos, sin, attention_mask):
    hidden_states = np.asarray(hidden_states, np.float32)
    prev_k = np.asarray(prev_k, np.float32)
    Wq = np.asarray(Wq, np.float32)
    Wk = np.asarray(Wk, np.float32)
    Wv = np.asarray(Wv, np.float32)
    Wo = np.asarray(Wo, np.float32)
    cos2d = np.asarray(cos, np.float32).reshape(L, HD)
    sin2d = np.asarray(sin, np.float32).reshape(L, HD)
    mask2d = np.asarray(attention_mask).reshape(L, L).astype(bool)

    classes, arb = _classify(mask2d)
    key = tuple(sorted(classes.items()))
    if key not in _PROGRAM_CACHE:
        _PROGRAM_CACHE[key] = build_program(classes, arb)
    nc = _PROGRAM_CACHE[key]

    # shared host-side constants
    sign = np.where(np.arange(128) % 2 == 0, -1.0, 1.0).astype(np.float32)
    d128 = np.concatenate([DPERM, DPERM])
    cs_full = np.empty((256, L), np.float16)
    cs_full[0:128] = cos2d[:, d128].T
    cs_full[128:256] = sin2d[:, d128].T * sign[:, None]
    id64 = np.eye(64, dtype=np.float32)
    maskf = None
    if arb:
        maskf = np.stack([
            np.ascontiguousarray(
                mask2d[qt * QT:(qt + 1) * QT, kt * KT:(kt + 1) * KT].T
            ).astype(np.float16)
            for (qt, kt) in arb])

    # per-batch hidden swizzle [128, NET*L]: hxA_b[p, et*L+l] = hidden[b,l,et*128+p]
    hxA = [np.ascontiguousarray(
        hidden_states[b].T.reshape(NET, 128, L).transpose(1, 0, 2)
        .reshape(128, NET * L)).astype(np.float16) for b in range(B)]

    # per-g weight swizzles (shared by cores g and g+4)
    order_q = [0, 1, 2, 3, 4, 5, 7, 6]               # pair tiles; mt3 = [j7 | j6]
    wq2s, wkv2s, wo2s = [], [], []
    for g in range(4):
        heads = [g + 4 * jj for jj in range(HPC)]
        wq_rows = np.concatenate([heads[jj] * HD + DPERM for jj in order_q])
        T = Wq[wq_rows, :]                           # [512, H] rows=(mt,m)
        wq2 = T.reshape(4, 128, NET, 128).transpose(0, 3, 2, 1).reshape(512, H)
        wq2s.append(wq2.astype(np.float16))
        U = np.concatenate([Wk[g * HD + DPERM, :], Wv[g * HD:(g + 1) * HD, :]],
                           axis=0)                   # [128, H] rows=m
        wkv2 = U.reshape(128, NET, 128).transpose(2, 1, 0).reshape(128, H)
        wkv2s.append(wkv2.astype(np.float16))
        wo_cols = np.concatenate(
            [np.arange(h * HD, (h + 1) * HD) for h in heads])
        wo2s.append(np.ascontiguousarray(Wo[:, wo_cols].T).astype(np.float16))

    in_maps = []
    for c in range(NCORES):
        b, g = c // 4, c % 4
        r = c % 4                                    # quad group rank
        pr = c // 4                                  # pair group rank
        heads = [g + 4 * jj for jj in range(HPC)]
        # pk[64j:64j+64, :] = prev_k[b, h_j][:, DPERM].T  (7 prev heads)
        pk_h = np.ascontiguousarray(
            prev_k[b][heads[:7]][:, :, DPERM].transpose(0, 2, 1)
            .reshape(448, L)).astype(np.float16)
        # pv[p, ((kt*7)+j)*65 + d] = prev_v[b, h_j, kt*128+p, d] (+ones col)
        pv_h = np.empty((NKT, 128, 7, HD + 1), np.float16)
        pvv = np.asarray(prev_v, np.float32)[b][heads[:7]].reshape(7, NKT, 128, HD)
        pv_h[:, :, :, :HD] = pvv.transpose(1, 2, 0, 3)
        pv_h[:, :, :, HD] = 1.0
        pv_h = np.ascontiguousarray(
            pv_h.transpose(1, 0, 2, 3).reshape(128, NKT * 7 * (HD + 1)))
        m = {
            "hx8": hxA[b][32 * r:32 * (r + 1), :],
            "wqh": wq2s[g][256 * pr:256 * (pr + 1), :],
            "wkvh": wkv2s[g][64 * pr:64 * (pr + 1), :],
            "woh": wo2s[g][256 * pr:256 * (pr + 1), :],
            "csh": cs_full[32 * c:32 * (c + 1), :],
            "pk": pk_h, "pv": pv_h, "id64": id64,
        }
        if arb:
            m["maskf"] = maskf
        in_maps.append(m)

    _LAST["nc"] = nc
    _LAST["in_maps"] = in_maps
    res = run_bass_kernel_spmd(nc, in_maps, list(range(NCORES)))
    out = np.zeros((B, L, H), np.float32)
    for c in range(NCORES):
        b, r = c // 4, c % 4
        out[b][:, 512 * r:512 * (r + 1)] = res.results[c]["outp"].T.astype(np.float32)
    return out
